# revision 28
# baseline (speedup 1.0000x reference)
# Trainium2 Bass kernel for nn_MultiHeadAttention_71674414235938
#
# MHA with a cross-modal additive bias gathered from a 3x3 table and a causal
# mask, B=1, S=2048, HID=1024, H=16 heads of D=64.
#
# Sharding: tensor-parallel over heads. 2 heads per core (dq slice of 128).
# Each core computes q/k/v projections for its heads, head-local attention,
# and a partial output ctx_c @ Wo[:, c*128:(c+1)*128].T which the host sums.
#
# Device-side layout choices:
#   * scores are computed TRANSPOSED: sT[j, i] = k[j]·q[i] (j on partitions),
#     so softmax-denominators and the attn@V contraction both run without any
#     on-chip transposes:  ctxT[d, i] = sum_j v'[j, d] * attnT[j, i]  with
#     lhsT = v' (natural layout) and rhs = attnT (as produced).
#   * the 3x3 cross-modal bias is rank-3:  bias = (onehot(m) @ cmw) @ onehot(m).T
#     so it is folded into the scores matmul by appending 3 rows (U.T to the
#     q side, R.T to the k side), K = 64+3 = 67.
#   * softmax runs without max-subtraction: scores are O(+-6) here, exp is
#     safely in fp32 range.
#   * a ones-column appended to v makes the PE accumulate the softmax
#     denominator into ctxT row 64; normalization happens on the way out of
#     PSUM (reciprocal + partition-broadcast DMA + multiply).
#   * causal structure: score blocks entirely above the diagonal are skipped;
#     diagonal staircase blocks are masked in-place with gpsimd affine_select.
#   * x is streamed n-major (512-column sequence chunks) so the q/k
#     projection chains start ~1us into the kernel instead of waiting for
#     the whole 4MiB x load; weights ride the scalar queue in parallel.
#   * the ic=3 context chains accumulate their first 12 j-blocks inside
#     phase 3, so the post-exp tail is only 4 matmuls per head + a
#     pipelined normalize + 4 out-projections on independent PSUM banks.

import math

import numpy as np
import ml_dtypes

B, S, HID, H, D = 1, 2048, 1024, 16, 64
NCORES = 8
HPC = H // NCORES          # heads per core = 2
DPC = HPC * D              # head-dim columns per core = 128
KC = HID // 128            # contraction chunks = 8
NIC = S // 512             # 512-wide i-chunks = 4
NJB = S // 128             # 128-tall j-blocks = 16

BF16 = ml_dtypes.bfloat16

_CACHE = {}


def _build_causal(has_bq: bool, has_bk: bool, has_bv: bool):
    from contextlib import ExitStack

    import concourse.bass as bass
    import concourse.bacc as bacc
    import concourse.mybir as mybir
    import concourse.tile as tile

    fp32 = mybir.dt.float32
    bf16 = mybir.dt.bfloat16
    Exp = mybir.ActivationFunctionType.Exp
    Copy = mybir.ActivationFunctionType.Copy

    nc = bacc.Bacc()

    # x pre-chunked n-major on the host: [NIC, 128, KC*512]
    xTn = nc.declare_dram_parameter("xTn", [NIC, 128, KC * 512], bf16,
                                    isOutput=False)
    wqT = nc.declare_dram_parameter("wqT", [HID, DPC], bf16, isOutput=False)
    wkT = nc.declare_dram_parameter("wkT", [HID, DPC], bf16, isOutput=False)
    wvT = nc.declare_dram_parameter("wvT", [HID, DPC], bf16, isOutput=False)
    woT = nc.declare_dram_parameter("woT", [DPC, HID], bf16, isOutput=False)
    uT = nc.declare_dram_parameter("uT", [4, S], bf16, isOutput=False)
    rT = nc.declare_dram_parameter("rT", [4, S], bf16, isOutput=False)
    if has_bq:
        bq = nc.declare_dram_parameter("bq", [DPC, 1], fp32, isOutput=False)
    if has_bk:
        bk = nc.declare_dram_parameter("bk", [DPC, 1], fp32, isOutput=False)
    if has_bv:
        bv = nc.declare_dram_parameter("bv", [1, DPC], fp32, isOutput=False)
    out = nc.declare_dram_parameter("out", [S, HID], bf16, isOutput=True)

    with tile.TileContext(nc) as tc, ExitStack() as ctx:
        pp = ctx.enter_context(tc.tile_pool(name="persist", bufs=1))

        # -- wq/wk first on sync (needed by the very first matmuls), then x
        #    n-major; low kc halves first so the first q/k matmuls start
        #    as soon as the first bytes land
        w_sbs = {}
        for nm, src in (("q", wqT), ("k", wkT)):
            w_sbs[nm] = pp.tile([128, KC, DPC], bf16, name=f"w{nm}_sb")
        xsb = pp.tile([128, NIC, KC, 512], bf16, name="xsb")
        xTn_re = xTn[:, :, :].rearrange("n p (kc c) -> n p kc c", kc=KC)
        for lo, hi in ((0, 4), (4, 8)):
            for nm, src in (("q", wqT), ("k", wkT)):
                src_re = src[:, :].rearrange("(kc p) m -> p kc m", p=128)
                nc.sync.dma_start(out=w_sbs[nm][:, lo:hi, :],
                                  in_=src_re[:, lo:hi, :])
            nc.sync.dma_start(out=xsb[:, 0, lo:hi, :],
                              in_=xTn_re[0, :, lo:hi, :])
        for n in range(1, NIC):
            nc.sync.dma_start(out=xsb[:, n, :, :], in_=xTn_re[n, :, :, :])
        # wv and the bias factor rows ride the scalar HWDGE queue in parallel
        w_sbs["v"] = pp.tile([128, KC, DPC], bf16, name="wv_sb")
        nc.scalar.dma_start(
            out=w_sbs["v"],
            in_=wvT[:, :].rearrange("(kc p) m -> p kc m", p=128),
        )

        # qU / kR: per head, 67 live rows ([0:64] proj, [64:67] bias factors)
        qU = [pp.tile([67, S], bf16, name=f"qU{h}") for h in range(HPC)]
        kR = [pp.tile([67, S], bf16, name=f"kR{h}") for h in range(HPC)]
        for h in range(HPC):
            nc.scalar.dma_start(out=qU[h][64:67, :], in_=uT[0:3, :])
            nc.scalar.dma_start(out=kR[h][64:67, :], in_=rT[0:3, :])
        # v': per j-block [128, 2 heads, 65] with ones in column 64
        vp = [pp.tile([128, HPC, 65], bf16, name=f"vp{jb}") for jb in range(NJB)]
        for jb in range(NJB):
            nc.gpsimd.memset(vp[jb][:, :, 64:65], 1.0)
        ones_sb = pp.tile([1, 64], fp32, name="ones_sb")
        nc.gpsimd.memset(ones_sb, 1.0)
        # wo is loaded late (see below) to keep the startup window for x
        wo_sb = pp.tile([128, HID], bf16)
        # normalized transposed context, both heads, one tile per i-chunk
        ctxT = [pp.tile([128, 512], bf16, name=f"ctxT{ic}") for ic in range(NIC)]
        if has_bq:
            bq_sb = pp.tile([DPC, 1], fp32)
            nc.gpsimd.dma_start(out=bq_sb, in_=bq[:, :])
        if has_bk:
            bk_sb = pp.tile([DPC, 1], fp32)
            nc.gpsimd.dma_start(out=bk_sb, in_=bk[:, :])
        if has_bv:
            bv_sb = pp.tile([128, DPC], fp32)
            bv_ap = bv[:, :]
            nc.gpsimd.dma_start(
                out=bv_sb,
                in_=bass.AP(tensor=bv_ap.tensor, offset=bv_ap.offset,
                            ap=[[0, 128], bv_ap.ap[1]]),
            )

        p2 = ctx.enter_context(tc.tile_pool(name="ph2", bufs=1))
        ps = ctx.enter_context(tc.tile_pool(name="ps", bufs=1, space="PSUM"))
        at_tiles = {}
        cps_tiles = {}

        def emit_qk_n(n, tq, tk):
            # one n-chunk of the q and k projections; copies split DVE/ACT
            pq = ps.tile([128, 512], fp32, tag=tq, name=f"psq{n}")
            pk = ps.tile([128, 512], fp32, tag=tk, name=f"psk{n}")
            for kc in range(KC):
                nc.tensor.matmul(pq, lhsT=w_sbs["q"][:, kc, :],
                                 rhs=xsb[:, n, kc, :],
                                 start=(kc == 0), stop=(kc == KC - 1))
                nc.tensor.matmul(pk, lhsT=w_sbs["k"][:, kc, :],
                                 rhs=xsb[:, n, kc, :],
                                 start=(kc == 0), stop=(kc == KC - 1))
            for nm, src_ps in (("q", pq), ("k", pk)):
                dsts = qU if nm == "q" else kR
                bias_sb = None
                if nm == "q" and has_bq:
                    bias_sb = bq_sb
                if nm == "k" and has_bk:
                    bias_sb = bk_sb
                for h in range(HPC):
                    dst = dsts[h][0:64, n * 512:(n + 1) * 512]
                    sr = src_ps[h * 64:(h + 1) * 64, :]
                    if bias_sb is not None:
                        nc.vector.tensor_scalar_add(
                            dst, sr, bias_sb[h * 64:(h + 1) * 64, 0:1]
                        )
                    elif h == 0:
                        nc.vector.tensor_copy(dst, sr)
                    else:
                        nc.scalar.activation(dst, sr, Copy)

        def emit_v(vjb, tag):
            psv = ps.tile([128, DPC], fp32, tag=tag, name=f"psv{vjb}")
            n, j0 = vjb // 4, (vjb % 4) * 128
            for kc in range(KC):
                nc.tensor.matmul(
                    psv,
                    lhsT=xsb[:, n, kc, j0:j0 + 128],
                    rhs=w_sbs["v"][:, kc, :],
                    start=(kc == 0), stop=(kc == KC - 1),
                )
            dst = vp[vjb][:, :, 0:64]
            sr = psv[:, :].rearrange("p (h m) -> p h m", h=HPC)
            if has_bv:
                bvr = bv_sb[:, :].rearrange("p (h m) -> p h m", h=HPC)
                nc.vector.tensor_add(dst, sr, bvr)
            else:
                nc.vector.tensor_copy(dst, sr)

        def emit_chunk(jb, ic):
            ics = (jb * 128) // 512
            w = S - ics * 512
            if jb not in at_tiles:
                at_tiles[jb] = p2.tile(
                    [128, HPC, w], bf16, tag=f"at{jb}", bufs=1,
                    name=f"at{jb}")
            at = at_tiles[jb]
            diag = ic == ics
            d0 = (jb % 4) * 128 if diag else 0
            sc = ps.tile([128, HPC * 512], fp32, tag="sc", bufs=2,
                         name=f"sc{jb}_{ic}")
            for h in range(HPC):
                nc.tensor.matmul(
                    sc[:, h * 512 + d0:(h + 1) * 512],
                    lhsT=kR[h][:, jb * 128:(jb + 1) * 128],
                    rhs=qU[h][:, ic * 512 + d0:(ic + 1) * 512],
                    start=True,
                    stop=True,
                )
            scr = sc[:, :].rearrange("p (h n) -> p h n", h=HPC)
            off = (ic - ics) * 512
            nc.scalar.activation(
                at[:, :, off + d0:off + 512], scr[:, :, d0:], Exp
            )
            if diag:
                # zero above-diagonal inside the 128-wide strip: keep f >= p
                # (columns left of d0 are never written OR read)
                nc.gpsimd.affine_select(
                    out=at[:, :, d0:d0 + 128], in_=at[:, :, d0:d0 + 128],
                    compare_op=mybir.AluOpType.is_ge,
                    fill=0.0, base=0,
                    pattern=[[0, HPC], [1, 128]],
                    channel_multiplier=-1,
                )

        def emit_ctx_mms(h, ic, jbs, first, last):
            key = (h, ic)
            if key not in cps_tiles:
                cps_tiles[key] = ps.tile([65, 512], fp32, tag="cd"[h],
                                         name=f"cps{h}_{ic}")
            cps = cps_tiles[key]
            for i, jb in enumerate(jbs):
                at = at_tiles[jb]
                ics = (jb * 128) // 512
                # diagonal blocks: skip the all-zero columns left of the
                # staircase (never written; masked out anyway)
                d0 = (jb % 4) * 128 if ics == ic else 0
                off = (ic - ics) * 512
                nc.tensor.matmul(
                    cps[:, d0:512],
                    lhsT=vp[jb][:, h, :],
                    rhs=at[:, h, off + d0:off + 512],
                    start=(first and i == 0),
                    stop=(last and i == len(jbs) - 1),
                )

        def emit_norm(h, ic):
            # den row -> SBUF, broadcast (gpsimd), reciprocal, scale ctx
            cps = cps_tiles[(h, ic)]
            rr = p2.tile([1, 512], fp32, tag="rr", bufs=2, name=f"rr{h}_{ic}")
            nc.vector.tensor_copy(rr, cps[64:65, :])
            rb = p2.tile([64, 512], fp32, tag="rb", bufs=2, name=f"rb{h}_{ic}")
            nc.gpsimd.partition_broadcast(rb, rr)
            nc.vector.reciprocal_approx_fast(rb, rb)
            nc.vector.tensor_mul(
                ctxT[ic][h * 64:(h + 1) * 64, :], cps[0:64, :], rb,
            )

        def emit_norm_pe(h, ic):
            # tail variant: broadcast via a tiny K=1 matmul so the PE stays
            # active (HAM stays un-throttled) and gpsimd latency is avoided
            cps = cps_tiles[(h, ic)]
            rr = p2.tile([1, 512], fp32, tag="rr", bufs=2, name=f"rrt{h}")
            nc.vector.tensor_copy(rr, cps[64:65, :])
            rbp = ps.tile([64, 512], fp32, tag="ab"[h], name=f"rbp{h}")
            nc.tensor.matmul(rbp, lhsT=ones_sb, rhs=rr, start=True, stop=True)
            rb = p2.tile([64, 512], fp32, tag="rb", bufs=2, name=f"rbt{h}")
            nc.vector.reciprocal_approx_fast(rb, rbp)
            nc.vector.tensor_mul(
                ctxT[ic][h * 64:(h + 1) * 64, :], cps[0:64, :], rb,
            )

        def emit_ctx(h, ic):
            emit_ctx_mms(h, ic, list(range(4 * (ic + 1))), True, True)
            emit_norm(h, ic)

        def emit_outproj(ib, tags=("a", "b"), copies="dve", dma_q=None):
            ob = p2.tile([128, HID], bf16, tag="ob", bufs=4, name=f"ob{ib}")
            wide = len(tags) == 1
            if wide:
                opsw = ps.tile([128, 1024], fp32, tag=tags[0], bufs=2,
                               name=f"opsw{ib}")
            for oc in range(2):
                if wide:
                    ops = opsw[:, oc * 512:(oc + 1) * 512]
                else:
                    ops = ps.tile([128, 512], fp32, tag=tags[oc],
                                  name=f"ops{ib}_{oc}")
                nc.tensor.matmul(
                    ops,
                    lhsT=ctxT[ib // 4][:, (ib % 4) * 128:(ib % 4 + 1) * 128],
                    rhs=wo_sb[:, oc * 512:(oc + 1) * 512],
                    start=True,
                    stop=True,
                )
                if wide:
                    continue  # single full-width copy below
                on_act = copies == "act" or (copies == "mix" and oc == 0)
                if on_act:
                    nc.scalar.activation(ob[:, oc * 512:(oc + 1) * 512],
                                         ops, Copy)
                else:
                    nc.vector.tensor_copy(ob[:, oc * 512:(oc + 1) * 512], ops)
            if wide:
                # one 1024-wide copy spanning both PSUM banks of the tile
                if copies == "act":
                    nc.scalar.activation(ob, opsw, Copy)
                else:
                    nc.vector.tensor_copy(ob, opsw)
            q = dma_q if dma_q is not None else nc.sync
            q.dma_start(out=out[ib * 128:(ib + 1) * 128, :], in_=ob)

        # ---- emission schedule ----
        # epoch 0: ordered to match the x n-group DMA arrival order, so the
        # PE never queues work whose inputs arrive later than other work
        emit_qk_n(0, "a", "b")
        emit_qk_n(1, "c", "d")
        for jb in range(4):
            emit_chunk(jb, 0)
        emit_v(0, "a")
        emit_v(1, "b")
        emit_v(2, "c")
        emit_v(3, "d")
        emit_qk_n(2, "a", "b")
        emit_v(4, "c")
        emit_v(5, "d")
        emit_qk_n(3, "c", "d")
        emit_v(6, "a")
        emit_v(7, "b")

        def interleave(ic, fillers):
            chunks = list(range(4 * (ic + 1)))
            fi = list(fillers)
            n_chunks = len(chunks)
            per = max(1, (n_chunks + len(fi) - 1) // max(1, len(fi)))
            while chunks or fi:
                for _ in range(per):
                    if chunks:
                        emit_chunk(chunks.pop(0), ic)
                if fi:
                    fi.pop(0)()

        # phase 1: 8 score chunks; fillers: last v chains + first ctx
        interleave(1, [
            lambda: emit_v(8, "c"), lambda: emit_ctx(0, 0),
            lambda: emit_v(9, "d"), lambda: emit_v(10, "a"),
            lambda: emit_ctx(1, 0), lambda: emit_v(11, "b"),
        ])
        # wo arrives well before op0 but stays out of the x startup window
        nc.gpsimd.dma_start(out=wo_sb, in_=woT[:, :])
        # phase 2: 12 chunks; ctx chains early so their normalize latency
        # hides under the chunk stream instead of stalling the next phase
        interleave(2, [
            lambda: emit_ctx(0, 1), lambda: emit_outproj(0),
            lambda: emit_v(12, "c"), lambda: emit_ctx(1, 1),
            lambda: emit_outproj(1), lambda: emit_v(13, "d"),
            lambda: emit_outproj(2), lambda: emit_v(14, "a"),
            lambda: emit_outproj(3), lambda: emit_v(15, "b"),
        ])
        # phase 3: 16 chunks; fillers end with the pure-PE ic=3 ctx prefix
        interleave(3, [
            lambda: emit_ctx(0, 2), lambda: emit_outproj(4),
            lambda: emit_outproj(5), lambda: emit_ctx(1, 2),
            lambda: emit_outproj(6), lambda: emit_outproj(7),
            lambda: emit_ctx_mms(0, 3, list(range(12)), True, False),
            lambda: emit_outproj(8),
            lambda: emit_ctx_mms(1, 3, list(range(12)), True, False),
            lambda: emit_outproj(9),
            lambda: emit_outproj(10),
            lambda: emit_outproj(11),
        ])
        # tail: finish ic=3 chains, PE-broadcast normalize, last 4 out-projs
        # on wide 2-bank PSUM tiles with single full-width copies
        emit_ctx_mms(0, 3, [12, 13, 14, 15], False, True)
        emit_norm_pe(0, 3)
        emit_ctx_mms(1, 3, [12, 13, 14, 15], False, True)
        emit_norm_pe(1, 3)
        emit_outproj(12, tags=("sc",), copies="act")
        emit_outproj(13, tags=("sc",), copies="dve", dma_q=nc.scalar)
        emit_outproj(14, tags=("sc",), copies="act")
        emit_outproj(15, tags=("sc",), copies="dve", dma_q=nc.scalar)

    nc.compile()
    return nc


def _build_noncausal(has_bq: bool, has_bk: bool, has_bv: bool):
    from contextlib import ExitStack

    import concourse.bass as bass
    import concourse.bacc as bacc
    import concourse.mybir as mybir
    import concourse.tile as tile

    fp32 = mybir.dt.float32
    bf16 = mybir.dt.bfloat16
    Exp = mybir.ActivationFunctionType.Exp

    nc = bacc.Bacc()

    xT = nc.declare_dram_parameter("xT", [HID, S], bf16, isOutput=False)
    wqT = nc.declare_dram_parameter("wqT", [HID, DPC], bf16, isOutput=False)
    wkT = nc.declare_dram_parameter("wkT", [HID, DPC], bf16, isOutput=False)
    wvT = nc.declare_dram_parameter("wvT", [HID, DPC], bf16, isOutput=False)
    woT = nc.declare_dram_parameter("woT", [DPC, HID], bf16, isOutput=False)
    uT = nc.declare_dram_parameter("uT", [4, S], bf16, isOutput=False)
    rT = nc.declare_dram_parameter("rT", [4, S], bf16, isOutput=False)
    if has_bq:
        bq = nc.declare_dram_parameter("bq", [DPC, 1], fp32, isOutput=False)
    if has_bk:
        bk = nc.declare_dram_parameter("bk", [DPC, 1], fp32, isOutput=False)
    if has_bv:
        bv = nc.declare_dram_parameter("bv", [1, DPC], fp32, isOutput=False)
    maskT = nc.declare_dram_parameter("maskT", [S, S], bf16, isOutput=False)
    out = nc.declare_dram_parameter("out", [S, HID], bf16, isOutput=True)

    with tile.TileContext(nc) as tc, ExitStack() as ctx:
        pp = ctx.enter_context(tc.tile_pool(name="persist", bufs=1))

        w_sbs = {}
        for nm, src in (("q", wqT), ("k", wkT)):
            w_sb = w_sbs[nm] = pp.tile([128, KC, DPC], bf16, name=f"w{nm}_sb")
            nc.sync.dma_start(
                out=w_sb, in_=src[:, :].rearrange("(kc p) m -> p kc m", p=128)
            )
        xT_re = xT[:, :].rearrange("(kc p) n -> p kc n", p=128)
        xT_sb = []
        for kc in range(KC):
            xk = pp.tile([128, S], bf16, name=f"xk{kc}")
            nc.sync.dma_start(out=xk, in_=xT_re[:, kc, :])
            xT_sb.append(xk)
        w_sbs["v"] = pp.tile([128, KC, DPC], bf16, name="wv_sb")
        nc.gpsimd.dma_start(
            out=w_sbs["v"],
            in_=wvT[:, :].rearrange("(kc p) m -> p kc m", p=128),
        )
        wo_sb = pp.tile([128, HID], bf16)
        nc.gpsimd.dma_start(out=wo_sb, in_=woT[:, :])

        qU = [pp.tile([67, S], bf16, name=f"qU{h}") for h in range(HPC)]
        kR = [pp.tile([67, S], bf16, name=f"kR{h}") for h in range(HPC)]
        for h in range(HPC):
            nc.gpsimd.dma_start(out=qU[h][64:67, :], in_=uT[0:3, :])
            nc.gpsimd.dma_start(out=kR[h][64:67, :], in_=rT[0:3, :])
        vp = [pp.tile([128, HPC, 65], bf16, name=f"vp{jb}") for jb in range(NJB)]
        for jb in range(NJB):
            nc.gpsimd.memset(vp[jb][:, :, 64:65], 1.0)
        ctxT = [pp.tile([128, 512], bf16, name=f"ctxT{ic}") for ic in range(NIC)]
        if has_bq:
            bq_sb = pp.tile([DPC, 1], fp32)
            nc.gpsimd.dma_start(out=bq_sb, in_=bq[:, :])
        if has_bk:
            bk_sb = pp.tile([DPC, 1], fp32)
            nc.gpsimd.dma_start(out=bk_sb, in_=bk[:, :])
        if has_bv:
            bv_sb = pp.tile([128, DPC], fp32)
            bv_ap = bv[:, :]
            nc.gpsimd.dma_start(
                out=bv_sb,
                in_=bass.AP(tensor=bv_ap.tensor, offset=bv_ap.offset,
                            ap=[[0, 128], bv_ap.ap[1]]),
            )

        p2 = ctx.enter_context(tc.tile_pool(name="ph2", bufs=1))
        ps = ctx.enter_context(tc.tile_pool(name="ps", bufs=1, space="PSUM"))
        at_tiles = {}
        vjb_iter = iter(range(NJB))

        def emit_qk_batch(ns, tags):
            chains = []
            for n in ns:
                for nm in ("q", "k"):
                    chains.append((nm, n))
            pqs = {}
            for (nm, n), tg in zip(chains, tags):
                pqs[(nm, n)] = ps.tile([128, 512], fp32, tag=tg,
                                       name=f"ps_{nm}{n}")
            for kc in range(KC):
                for nm, n in chains:
                    nc.tensor.matmul(
                        pqs[(nm, n)],
                        lhsT=w_sbs[nm][:, kc, :],
                        rhs=xT_sb[kc][:, n * 512:(n + 1) * 512],
                        start=(kc == 0),
                        stop=(kc == KC - 1),
                    )
            for nm, n in chains:
                dsts = qU if nm == "q" else kR
                bias_sb = None
                if nm == "q" and has_bq:
                    bias_sb = bq_sb
                if nm == "k" and has_bk:
                    bias_sb = bk_sb
                for h in range(HPC):
                    dst = dsts[h][0:64, n * 512:(n + 1) * 512]
                    sr = pqs[(nm, n)][h * 64:(h + 1) * 64, :]
                    if bias_sb is not None:
                        nc.vector.tensor_scalar_add(
                            dst, sr, bias_sb[h * 64:(h + 1) * 64, 0:1]
                        )
                    else:
                        nc.vector.tensor_copy(dst, sr)

        def emit_v(count):
            for vjb in [v for _, v in zip(range(count), vjb_iter)]:
                psv = ps.tile([128, DPC], fp32, tag=f"abcd"[vjb % 4],
                              name=f"psv{vjb}")
                for kc in range(KC):
                    nc.tensor.matmul(
                        psv,
                        lhsT=xT_sb[kc][:, vjb * 128:(vjb + 1) * 128],
                        rhs=w_sbs["v"][:, kc, :],
                        start=(kc == 0),
                        stop=(kc == KC - 1),
                    )
                dst = vp[vjb][:, :, 0:64]
                sr = psv[:, :].rearrange("p (h m) -> p h m", h=HPC)
                if has_bv:
                    bvr = bv_sb[:, :].rearrange("p (h m) -> p h m", h=HPC)
                    nc.vector.tensor_add(dst, sr, bvr)
                else:
                    nc.vector.tensor_copy(dst, sr)

        def emit_chunk(jb, ic):
            key = (jb, ic)
            at_tiles[key] = p2.tile(
                [128, HPC, 512], bf16, tag=f"at{jb}", bufs=2,
                name=f"at{jb}_{ic}")
            at = at_tiles[key]
            sc = ps.tile([128, HPC * 512], fp32, tag="sc", bufs=2,
                         name=f"sc{jb}_{ic}")
            for h in range(HPC):
                nc.tensor.matmul(
                    sc[:, h * 512:(h + 1) * 512],
                    lhsT=kR[h][:, jb * 128:(jb + 1) * 128],
                    rhs=qU[h][:, ic * 512:(ic + 1) * 512],
                    start=True,
                    stop=True,
                )
            scr = sc[:, :].rearrange("p (h n) -> p h n", h=HPC)
            nc.scalar.activation(at, scr, Exp)
            mt = p2.tile([128, 512], bf16, tag="mt", bufs=2,
                         name=f"mt{jb}_{ic}")
            nc.sync.dma_start(
                out=mt,
                in_=maskT[jb * 128:(jb + 1) * 128,
                          ic * 512:(ic + 1) * 512])
            mt_b2 = bass.AP(
                tensor=mt.tensor, offset=mt.offset,
                ap=[mt.ap[0], [0, HPC], mt.ap[1]],
            )
            nc.vector.tensor_mul(at, at, mt_b2)

        def emit_ctx(h, ic):
            jmax = NJB
            cps = ps.tile([65, 512], fp32, tag="ab"[h], name=f"cps{h}_{ic}")
            for jb in range(jmax):
                rhs = at_tiles[(jb, ic)][:, h, 0:512]
                nc.tensor.matmul(
                    cps,
                    lhsT=vp[jb][:, h, :],
                    rhs=rhs,
                    start=(jb == 0),
                    stop=(jb == jmax - 1),
                )
            rr = p2.tile([1, 512], fp32, tag="rr", bufs=2, name=f"rr{h}_{ic}")
            nc.vector.tensor_copy(rr, cps[64:65, :])
            rb = p2.tile([64, 512], fp32, tag="rb", bufs=2, name=f"rb{h}_{ic}")
            nc.gpsimd.partition_broadcast(rb, rr)
            nc.vector.reciprocal_approx_fast(rb, rb)
            nc.vector.tensor_mul(
                ctxT[ic][h * 64:(h + 1) * 64, :], cps[0:64, :], rb,
            )

        def emit_outproj(ib):
            ob = p2.tile([128, HID], bf16, tag="ob", bufs=3, name=f"ob{ib}")
            for oc in range(2):
                ops = ps.tile([128, 512], fp32, tag="cd"[oc],
                              name=f"ops{ib}_{oc}")
                nc.tensor.matmul(
                    ops,
                    lhsT=ctxT[ib // 4][:, (ib % 4) * 128:(ib % 4 + 1) * 128],
                    rhs=wo_sb[:, oc * 512:(oc + 1) * 512],
                    start=True,
                    stop=True,
                )
                nc.vector.tensor_copy(ob[:, oc * 512:(oc + 1) * 512], ops)
            nc.sync.dma_start(out=out[ib * 128:(ib + 1) * 128, :], in_=ob)

        emit_qk_batch([0, 1], ["a", "b", "c", "d"])
        emit_qk_batch([2, 3], ["a", "b", "c", "d"])
        emit_v(NJB)
        for ic in range(NIC):
            for jb in range(NJB):
                emit_chunk(jb, ic)
            for h in range(HPC):
                emit_ctx(h, ic)
            for ib in range(4 * ic, 4 * (ic + 1)):
                emit_outproj(ib)

    nc.compile()
    return nc



def kernel(x, Wq, bq, Wk, bk, Wv, bv, Wo, bo, cmw, mask, modality_info,
           _perf=None):
    from concourse.bass_utils import run_bass_kernel_spmd

    x = np.asarray(x, np.float32)
    Wq = np.asarray(Wq, np.float32)
    Wk = np.asarray(Wk, np.float32)
    Wv = np.asarray(Wv, np.float32)
    Wo = np.asarray(Wo, np.float32)
    bq_ = np.asarray(bq, np.float32)
    bk_ = np.asarray(bk, np.float32)
    bv_ = np.asarray(bv, np.float32)
    bo_ = np.asarray(bo, np.float32)
    cmw = np.asarray(cmw, np.float32)
    mask2 = np.asarray(mask)[0]
    mi = np.asarray(modality_info).astype(np.int64)[0]

    causal = bool(
        np.array_equal(mask2 != 0, np.tril(np.ones((S, S), bool)))
    )
    has_bq = bool(np.any(bq_))
    has_bk = bool(np.any(bk_))
    has_bv = bool(np.any(bv_))

    key = (causal, has_bq, has_bk, has_bv)
    if key not in _CACHE:
        if causal:
            _CACHE[key] = _build_causal(has_bq, has_bk, has_bv)
        else:
            _CACHE[key] = _build_noncausal(has_bq, has_bk, has_bv)
    nc = _CACHE[key]

    scale = 1.0 / math.sqrt(D)
    # rank-3 factorization of the gathered cross-modal bias
    R = np.zeros((S, 3), np.float32)
    R[np.arange(S), mi] = 1.0
    U = R @ cmw
    uT4 = np.zeros((4, S), BF16)
    rT4 = np.zeros((4, S), BF16)
    uT4[0:3, :] = U.T.astype(BF16)
    rT4[0:3, :] = R.T.astype(BF16)
    xTb = np.ascontiguousarray(x[0].T).astype(BF16)
    if causal:
        # n-major chunks: [NIC, 128, KC*512]
        xTn = np.ascontiguousarray(
            xTb.reshape(KC, 128, NIC, 512).transpose(2, 1, 0, 3)
            .reshape(NIC, 128, KC * 512))

    in_maps = []
    for c in range(NCORES):
        sl = slice(c * DPC, (c + 1) * DPC)
        m = {
            # scores scale folded into the q-side weights (and bias)
            "wqT": np.ascontiguousarray(Wq[sl, :].T * scale).astype(BF16),
            "wkT": np.ascontiguousarray(Wk[sl, :].T).astype(BF16),
            "wvT": np.ascontiguousarray(Wv[sl, :].T).astype(BF16),
            "woT": np.ascontiguousarray(Wo[:, sl].T).astype(BF16),
            "uT": uT4,
            "rT": rT4,
        }
        if causal:
            m["xTn"] = xTn
        else:
            m["xT"] = xTb
        if has_bq:
            m["bq"] = np.ascontiguousarray(bq_[sl, None] * scale)
        if has_bk:
            m["bk"] = np.ascontiguousarray(bk_[sl, None])
        if has_bv:
            m["bv"] = np.ascontiguousarray(bv_[None, sl])
        if not causal:
            m["maskT"] = np.ascontiguousarray(mask2.T != 0).astype(BF16)
        in_maps.append(m)

    res = run_bass_kernel_spmd(
        nc, in_maps, core_ids=list(range(NCORES)),
        trace=bool(_perf is not None),
    )
    outp = np.zeros((S, HID), np.float32)
    for r in res.results:
        outp += np.asarray(r["out"], dtype=np.float32)
    outp += bo_[None, :]
    if _perf is not None:
        _perf["exec_time_ns"] = res.exec_time_ns
        _perf["trace"] = res.instructions_and_trace
    return outp.reshape(B, S, HID)


# revision 29
# speedup vs baseline: 1.0299x; 1.0299x over previous
# Trainium2 Bass kernel for nn_MultiHeadAttention_71674414235938
#
# MHA with a cross-modal additive bias gathered from a 3x3 table and a causal
# mask, B=1, S=2048, HID=1024, H=16 heads of D=64.
#
# Sharding: tensor-parallel over heads. 2 heads per core (dq slice of 128).
# Each core computes q/k/v projections for its heads, head-local attention,
# and a partial output ctx_c @ Wo[:, c*128:(c+1)*128].T which the host sums.
#
# Device-side layout choices:
#   * scores are computed TRANSPOSED: sT[j, i] = k[j]·q[i] (j on partitions),
#     so softmax-denominators and the attn@V contraction both run without any
#     on-chip transposes:  ctxT[d, i] = sum_j v'[j, d] * attnT[j, i]  with
#     lhsT = v' (natural layout) and rhs = attnT (as produced).
#   * the 3x3 cross-modal bias is rank-3:  bias = (onehot(m) @ cmw) @ onehot(m).T
#     so it is folded into the scores matmul by appending 3 rows (U.T to the
#     q side, R.T to the k side), K = 64+3 = 67.
#   * softmax runs without max-subtraction: scores are O(+-6) here, exp is
#     safely in fp32 range.
#   * a ones-column appended to v makes the PE accumulate the softmax
#     denominator into ctxT row 64; normalization happens on the way out of
#     PSUM (reciprocal + partition-broadcast DMA + multiply).
#   * causal structure: score blocks entirely above the diagonal are skipped;
#     diagonal staircase blocks are masked in-place with gpsimd affine_select.
#   * x is streamed n-major (512-column sequence chunks) so the q/k
#     projection chains start ~1us into the kernel instead of waiting for
#     the whole 4MiB x load; weights ride the scalar queue in parallel.
#   * the ic=3 context chains accumulate their first 12 j-blocks inside
#     phase 3, so the post-exp tail is only 4 matmuls per head + a
#     pipelined normalize + 4 out-projections on independent PSUM banks.

import math

import numpy as np
import ml_dtypes

B, S, HID, H, D = 1, 2048, 1024, 16, 64
NCORES = 8
HPC = H // NCORES          # heads per core = 2
DPC = HPC * D              # head-dim columns per core = 128
KC = HID // 128            # contraction chunks = 8
NIC = S // 512             # 512-wide i-chunks = 4
NJB = S // 128             # 128-tall j-blocks = 16

BF16 = ml_dtypes.bfloat16

_CACHE = {}


def _build_causal(has_bq: bool, has_bk: bool, has_bv: bool):
    from contextlib import ExitStack

    import concourse.bass as bass
    import concourse.bacc as bacc
    import concourse.mybir as mybir
    import concourse.tile as tile

    fp32 = mybir.dt.float32
    bf16 = mybir.dt.bfloat16
    Exp = mybir.ActivationFunctionType.Exp
    Copy = mybir.ActivationFunctionType.Copy

    nc = bacc.Bacc()

    # x pre-chunked n-major on the host: [NIC, 128, KC*512]
    xTn = nc.declare_dram_parameter("xTn", [NIC, 128, KC * 512], bf16,
                                    isOutput=False)
    wqT = nc.declare_dram_parameter("wqT", [HID, DPC], bf16, isOutput=False)
    wkT = nc.declare_dram_parameter("wkT", [HID, DPC], bf16, isOutput=False)
    wvT = nc.declare_dram_parameter("wvT", [HID, DPC], bf16, isOutput=False)
    woT = nc.declare_dram_parameter("woT", [DPC, HID], bf16, isOutput=False)
    uT = nc.declare_dram_parameter("uT", [4, S], bf16, isOutput=False)
    rT = nc.declare_dram_parameter("rT", [4, S], bf16, isOutput=False)
    if has_bq:
        bq = nc.declare_dram_parameter("bq", [DPC, 1], fp32, isOutput=False)
    if has_bk:
        bk = nc.declare_dram_parameter("bk", [DPC, 1], fp32, isOutput=False)
    if has_bv:
        bv = nc.declare_dram_parameter("bv", [1, DPC], fp32, isOutput=False)
    out = nc.declare_dram_parameter("out", [S, HID], bf16, isOutput=True)

    with tile.TileContext(nc) as tc, ExitStack() as ctx:
        pp = ctx.enter_context(tc.tile_pool(name="persist", bufs=1))

        # -- wq/wk first on sync (needed by the very first matmuls), then x
        #    n-major; low kc halves first so the first q/k matmuls start
        #    as soon as the first bytes land
        w_sbs = {}
        for nm, src in (("q", wqT), ("k", wkT)):
            w_sbs[nm] = pp.tile([128, KC, DPC], bf16, name=f"w{nm}_sb")
        xsb = pp.tile([128, NIC, KC, 512], bf16, name="xsb")
        xTn_re = xTn[:, :, :].rearrange("n p (kc c) -> n p kc c", kc=KC)
        for lo, hi in ((0, 4), (4, 8)):
            for nm, src in (("q", wqT), ("k", wkT)):
                src_re = src[:, :].rearrange("(kc p) m -> p kc m", p=128)
                nc.sync.dma_start(out=w_sbs[nm][:, lo:hi, :],
                                  in_=src_re[:, lo:hi, :])
            nc.sync.dma_start(out=xsb[:, 0, lo:hi, :],
                              in_=xTn_re[0, :, lo:hi, :])
        for n in range(1, NIC):
            nc.sync.dma_start(out=xsb[:, n, :, :], in_=xTn_re[n, :, :, :])
        # wv and the bias factor rows ride the scalar HWDGE queue in parallel
        w_sbs["v"] = pp.tile([128, KC, DPC], bf16, name="wv_sb")
        nc.scalar.dma_start(
            out=w_sbs["v"],
            in_=wvT[:, :].rearrange("(kc p) m -> p kc m", p=128),
        )

        # qU / kR: per head, 67 live rows ([0:64] proj, [64:67] bias factors)
        qU = [pp.tile([67, S], bf16, name=f"qU{h}") for h in range(HPC)]
        kR = [pp.tile([67, S], bf16, name=f"kR{h}") for h in range(HPC)]
        for h in range(HPC):
            nc.scalar.dma_start(out=qU[h][64:67, :], in_=uT[0:3, :])
            nc.scalar.dma_start(out=kR[h][64:67, :], in_=rT[0:3, :])
        # v': per j-block [128, 2 heads, 65] with ones in column 64
        vp = [pp.tile([128, HPC, 65], bf16, name=f"vp{jb}") for jb in range(NJB)]
        for jb in range(NJB):
            nc.gpsimd.memset(vp[jb][:, :, 64:65], 1.0)
        ones_sb = pp.tile([1, 64], fp32, name="ones_sb")
        nc.gpsimd.memset(ones_sb, 1.0)
        # wo is loaded late (see below) to keep the startup window for x
        wo_sb = pp.tile([128, HID], bf16)
        # normalized transposed context, both heads, one tile per i-chunk
        ctxT = [pp.tile([128, 512], bf16, name=f"ctxT{ic}") for ic in range(NIC)]
        if has_bq:
            bq_sb = pp.tile([DPC, 1], fp32)
            nc.gpsimd.dma_start(out=bq_sb, in_=bq[:, :])
        if has_bk:
            bk_sb = pp.tile([DPC, 1], fp32)
            nc.gpsimd.dma_start(out=bk_sb, in_=bk[:, :])
        if has_bv:
            bv_sb = pp.tile([128, DPC], fp32)
            bv_ap = bv[:, :]
            nc.gpsimd.dma_start(
                out=bv_sb,
                in_=bass.AP(tensor=bv_ap.tensor, offset=bv_ap.offset,
                            ap=[[0, 128], bv_ap.ap[1]]),
            )

        p2 = ctx.enter_context(tc.tile_pool(name="ph2", bufs=1))
        ps = ctx.enter_context(tc.tile_pool(name="ps", bufs=1, space="PSUM"))
        at_tiles = {}
        cps_tiles = {}

        def emit_qk_n(n, tq, tk):
            # one n-chunk of the q and k projections; copies split DVE/ACT
            pq = ps.tile([128, 512], fp32, tag=tq, name=f"psq{n}")
            pk = ps.tile([128, 512], fp32, tag=tk, name=f"psk{n}")
            for kc in range(KC):
                nc.tensor.matmul(pq, lhsT=w_sbs["q"][:, kc, :],
                                 rhs=xsb[:, n, kc, :],
                                 start=(kc == 0), stop=(kc == KC - 1))
                nc.tensor.matmul(pk, lhsT=w_sbs["k"][:, kc, :],
                                 rhs=xsb[:, n, kc, :],
                                 start=(kc == 0), stop=(kc == KC - 1))
            for nm, src_ps in (("q", pq), ("k", pk)):
                dsts = qU if nm == "q" else kR
                bias_sb = None
                if nm == "q" and has_bq:
                    bias_sb = bq_sb
                if nm == "k" and has_bk:
                    bias_sb = bk_sb
                for h in range(HPC):
                    dst = dsts[h][0:64, n * 512:(n + 1) * 512]
                    sr = src_ps[h * 64:(h + 1) * 64, :]
                    if bias_sb is not None:
                        nc.vector.tensor_scalar_add(
                            dst, sr, bias_sb[h * 64:(h + 1) * 64, 0:1]
                        )
                    elif h == 0:
                        nc.vector.tensor_copy(dst, sr)
                    else:
                        nc.scalar.activation(dst, sr, Copy)

        def emit_v(vjb, tag):
            psv = ps.tile([128, DPC], fp32, tag=tag, name=f"psv{vjb}")
            n, j0 = vjb // 4, (vjb % 4) * 128
            for kc in range(KC):
                nc.tensor.matmul(
                    psv,
                    lhsT=xsb[:, n, kc, j0:j0 + 128],
                    rhs=w_sbs["v"][:, kc, :],
                    start=(kc == 0), stop=(kc == KC - 1),
                )
            dst = vp[vjb][:, :, 0:64]
            sr = psv[:, :].rearrange("p (h m) -> p h m", h=HPC)
            if has_bv:
                bvr = bv_sb[:, :].rearrange("p (h m) -> p h m", h=HPC)
                nc.vector.tensor_add(dst, sr, bvr)
            else:
                nc.vector.tensor_copy(dst, sr)

        def emit_chunk(jb, ic):
            ics = (jb * 128) // 512
            w = S - ics * 512
            if jb not in at_tiles:
                at_tiles[jb] = p2.tile(
                    [128, HPC, w], bf16, tag=f"at{jb}", bufs=1,
                    name=f"at{jb}")
            at = at_tiles[jb]
            diag = ic == ics
            d0 = (jb % 4) * 128 if diag else 0
            sc = ps.tile([128, HPC * 512], fp32, tag="sc", bufs=2,
                         name=f"sc{jb}_{ic}")
            for h in range(HPC):
                nc.tensor.matmul(
                    sc[:, h * 512 + d0:(h + 1) * 512],
                    lhsT=kR[h][:, jb * 128:(jb + 1) * 128],
                    rhs=qU[h][:, ic * 512 + d0:(ic + 1) * 512],
                    start=True,
                    stop=True,
                )
            scr = sc[:, :].rearrange("p (h n) -> p h n", h=HPC)
            off = (ic - ics) * 512
            nc.scalar.activation(
                at[:, :, off + d0:off + 512], scr[:, :, d0:], Exp
            )
            if diag:
                # zero above-diagonal inside the 128-wide strip: keep f >= p
                # (columns left of d0 are never written OR read)
                nc.gpsimd.affine_select(
                    out=at[:, :, d0:d0 + 128], in_=at[:, :, d0:d0 + 128],
                    compare_op=mybir.AluOpType.is_ge,
                    fill=0.0, base=0,
                    pattern=[[0, HPC], [1, 128]],
                    channel_multiplier=-1,
                )

        def emit_ctx_mms(h, ic, jbs, first, last):
            key = (h, ic)
            if key not in cps_tiles:
                cps_tiles[key] = ps.tile([65, 512], fp32, tag="cd"[h],
                                         name=f"cps{h}_{ic}")
            cps = cps_tiles[key]
            for i, jb in enumerate(jbs):
                at = at_tiles[jb]
                ics = (jb * 128) // 512
                # diagonal blocks: skip the all-zero columns left of the
                # staircase (never written; masked out anyway)
                d0 = (jb % 4) * 128 if ics == ic else 0
                off = (ic - ics) * 512
                nc.tensor.matmul(
                    cps[:, d0:512],
                    lhsT=vp[jb][:, h, :],
                    rhs=at[:, h, off + d0:off + 512],
                    start=(first and i == 0),
                    stop=(last and i == len(jbs) - 1),
                )

        def emit_norm(h, ic):
            # den row -> SBUF, broadcast (gpsimd), reciprocal, scale ctx
            cps = cps_tiles[(h, ic)]
            rr = p2.tile([1, 512], fp32, tag="rr", bufs=2, name=f"rr{h}_{ic}")
            nc.vector.tensor_copy(rr, cps[64:65, :])
            rb = p2.tile([64, 512], fp32, tag="rb", bufs=2, name=f"rb{h}_{ic}")
            nc.gpsimd.partition_broadcast(rb, rr)
            nc.vector.reciprocal_approx_fast(rb, rb)
            nc.vector.tensor_mul(
                ctxT[ic][h * 64:(h + 1) * 64, :], cps[0:64, :], rb,
            )

        def emit_norm_pe(h, ic):
            # tail variant: broadcast via a tiny K=1 matmul so the PE stays
            # active (HAM stays un-throttled) and gpsimd latency is avoided
            cps = cps_tiles[(h, ic)]
            rr = p2.tile([1, 512], fp32, tag="rr", bufs=2, name=f"rrt{h}")
            nc.vector.tensor_copy(rr, cps[64:65, :])
            rbp = ps.tile([64, 512], fp32, tag="ab"[h], name=f"rbp{h}")
            nc.tensor.matmul(rbp, lhsT=ones_sb, rhs=rr, start=True, stop=True)
            rb = p2.tile([64, 512], fp32, tag="rb", bufs=2, name=f"rbt{h}")
            nc.vector.reciprocal_approx_fast(rb, rbp)
            nc.vector.tensor_mul(
                ctxT[ic][h * 64:(h + 1) * 64, :], cps[0:64, :], rb,
            )

        def emit_ctx(h, ic):
            emit_ctx_mms(h, ic, list(range(4 * (ic + 1))), True, True)
            emit_norm(h, ic)

        def emit_outproj(ib, tags=("a", "b"), copies="dve", dma_q=None):
            ob = p2.tile([128, HID], bf16, tag="ob", bufs=4, name=f"ob{ib}")
            wide = len(tags) == 1
            if wide:
                opsw = ps.tile([128, 1024], fp32, tag=tags[0], bufs=2,
                               name=f"opsw{ib}")
            for oc in range(2):
                if wide:
                    ops = opsw[:, oc * 512:(oc + 1) * 512]
                else:
                    ops = ps.tile([128, 512], fp32, tag=tags[oc],
                                  name=f"ops{ib}_{oc}")
                nc.tensor.matmul(
                    ops,
                    lhsT=ctxT[ib // 4][:, (ib % 4) * 128:(ib % 4 + 1) * 128],
                    rhs=wo_sb[:, oc * 512:(oc + 1) * 512],
                    start=True,
                    stop=True,
                )
                if wide:
                    continue  # single full-width copy below
                on_act = copies == "act" or (copies == "mix" and oc == 0)
                if on_act:
                    nc.scalar.activation(ob[:, oc * 512:(oc + 1) * 512],
                                         ops, Copy)
                else:
                    nc.vector.tensor_copy(ob[:, oc * 512:(oc + 1) * 512], ops)
            if wide:
                # one 1024-wide copy spanning both PSUM banks of the tile
                if copies == "act":
                    nc.scalar.activation(ob, opsw, Copy)
                else:
                    nc.vector.tensor_copy(ob, opsw)
            q = dma_q if dma_q is not None else nc.sync
            q.dma_start(out=out[ib * 128:(ib + 1) * 128, :], in_=ob)

        # ---- emission schedule ----
        # epoch 0: ordered to match the x n-group DMA arrival order, so the
        # PE never queues work whose inputs arrive later than other work
        emit_qk_n(0, "a", "b")
        emit_qk_n(1, "c", "d")
        for jb in range(4):
            emit_chunk(jb, 0)
        emit_v(0, "a")
        emit_v(1, "b")
        emit_v(2, "c")
        emit_v(3, "d")
        emit_qk_n(2, "a", "b")
        emit_v(4, "c")
        emit_v(5, "d")
        emit_qk_n(3, "c", "d")
        emit_v(6, "a")
        emit_v(7, "b")

        def interleave(ic, fillers):
            chunks = list(range(4 * (ic + 1)))
            fi = list(fillers)
            n_chunks = len(chunks)
            per = max(1, (n_chunks + len(fi) - 1) // max(1, len(fi)))
            while chunks or fi:
                for _ in range(per):
                    if chunks:
                        emit_chunk(chunks.pop(0), ic)
                if fi:
                    fi.pop(0)()

        # phase 1: 8 score chunks; fillers: last v chains + first ctx
        interleave(1, [
            lambda: emit_v(8, "c"), lambda: emit_ctx(0, 0),
            lambda: emit_v(9, "d"), lambda: emit_v(10, "a"),
            lambda: emit_ctx(1, 0), lambda: emit_v(11, "b"),
        ])
        # wo arrives well before op0 but stays out of the x startup window
        nc.gpsimd.dma_start(out=wo_sb, in_=woT[:, :])
        # phase 2: 12 chunks; ctx chains early so their normalize latency
        # hides under the chunk stream instead of stalling the next phase
        interleave(2, [
            lambda: emit_ctx(0, 1), lambda: emit_outproj(0),
            lambda: emit_v(12, "c"), lambda: emit_ctx(1, 1),
            lambda: emit_outproj(1), lambda: emit_v(13, "d"),
            lambda: emit_outproj(2), lambda: emit_v(14, "a"),
            lambda: emit_outproj(3), lambda: emit_v(15, "b"),
        ])
        # phase 3: 16 chunks; fillers end with the pure-PE ic=3 ctx prefix
        interleave(3, [
            lambda: emit_ctx(0, 2), lambda: emit_outproj(4),
            lambda: emit_outproj(5), lambda: emit_ctx(1, 2),
            lambda: emit_outproj(6), lambda: emit_outproj(7),
            lambda: emit_ctx_mms(0, 3, list(range(12)), True, False),
            lambda: emit_outproj(8),
            lambda: emit_ctx_mms(1, 3, list(range(12)), True, False),
            lambda: emit_outproj(9),
        ])
        # tail: finish ic=3 chains, PE-broadcast normalize, last 6 out-projs
        # on a deep PSUM rotation so matmuls never wait on copies
        emit_ctx_mms(0, 3, [12, 13, 14, 15], False, True)
        emit_norm_pe(0, 3)
        emit_ctx_mms(1, 3, [12, 13, 14, 15], False, True)
        emit_norm_pe(1, 3)
        emit_outproj(10, tags=("a", "b"), copies="mix")
        emit_outproj(11, tags=("c", "d"), copies="mix", dma_q=nc.scalar)
        emit_outproj(12, tags=("sc",), copies="act")
        emit_outproj(13, tags=("sc",), copies="dve", dma_q=nc.scalar)
        emit_outproj(14, tags=("a", "b"), copies="mix")
        emit_outproj(15, tags=("c", "d"), copies="mix", dma_q=nc.scalar)

    nc.compile()
    return nc


def _build_noncausal(has_bq: bool, has_bk: bool, has_bv: bool):
    from contextlib import ExitStack

    import concourse.bass as bass
    import concourse.bacc as bacc
    import concourse.mybir as mybir
    import concourse.tile as tile

    fp32 = mybir.dt.float32
    bf16 = mybir.dt.bfloat16
    Exp = mybir.ActivationFunctionType.Exp

    nc = bacc.Bacc()

    xT = nc.declare_dram_parameter("xT", [HID, S], bf16, isOutput=False)
    wqT = nc.declare_dram_parameter("wqT", [HID, DPC], bf16, isOutput=False)
    wkT = nc.declare_dram_parameter("wkT", [HID, DPC], bf16, isOutput=False)
    wvT = nc.declare_dram_parameter("wvT", [HID, DPC], bf16, isOutput=False)
    woT = nc.declare_dram_parameter("woT", [DPC, HID], bf16, isOutput=False)
    uT = nc.declare_dram_parameter("uT", [4, S], bf16, isOutput=False)
    rT = nc.declare_dram_parameter("rT", [4, S], bf16, isOutput=False)
    if has_bq:
        bq = nc.declare_dram_parameter("bq", [DPC, 1], fp32, isOutput=False)
    if has_bk:
        bk = nc.declare_dram_parameter("bk", [DPC, 1], fp32, isOutput=False)
    if has_bv:
        bv = nc.declare_dram_parameter("bv", [1, DPC], fp32, isOutput=False)
    maskT = nc.declare_dram_parameter("maskT", [S, S], bf16, isOutput=False)
    out = nc.declare_dram_parameter("out", [S, HID], bf16, isOutput=True)

    with tile.TileContext(nc) as tc, ExitStack() as ctx:
        pp = ctx.enter_context(tc.tile_pool(name="persist", bufs=1))

        w_sbs = {}
        for nm, src in (("q", wqT), ("k", wkT)):
            w_sb = w_sbs[nm] = pp.tile([128, KC, DPC], bf16, name=f"w{nm}_sb")
            nc.sync.dma_start(
                out=w_sb, in_=src[:, :].rearrange("(kc p) m -> p kc m", p=128)
            )
        xT_re = xT[:, :].rearrange("(kc p) n -> p kc n", p=128)
        xT_sb = []
        for kc in range(KC):
            xk = pp.tile([128, S], bf16, name=f"xk{kc}")
            nc.sync.dma_start(out=xk, in_=xT_re[:, kc, :])
            xT_sb.append(xk)
        w_sbs["v"] = pp.tile([128, KC, DPC], bf16, name="wv_sb")
        nc.gpsimd.dma_start(
            out=w_sbs["v"],
            in_=wvT[:, :].rearrange("(kc p) m -> p kc m", p=128),
        )
        wo_sb = pp.tile([128, HID], bf16)
        nc.gpsimd.dma_start(out=wo_sb, in_=woT[:, :])

        qU = [pp.tile([67, S], bf16, name=f"qU{h}") for h in range(HPC)]
        kR = [pp.tile([67, S], bf16, name=f"kR{h}") for h in range(HPC)]
        for h in range(HPC):
            nc.gpsimd.dma_start(out=qU[h][64:67, :], in_=uT[0:3, :])
            nc.gpsimd.dma_start(out=kR[h][64:67, :], in_=rT[0:3, :])
        vp = [pp.tile([128, HPC, 65], bf16, name=f"vp{jb}") for jb in range(NJB)]
        for jb in range(NJB):
            nc.gpsimd.memset(vp[jb][:, :, 64:65], 1.0)
        ctxT = [pp.tile([128, 512], bf16, name=f"ctxT{ic}") for ic in range(NIC)]
        if has_bq:
            bq_sb = pp.tile([DPC, 1], fp32)
            nc.gpsimd.dma_start(out=bq_sb, in_=bq[:, :])
        if has_bk:
            bk_sb = pp.tile([DPC, 1], fp32)
            nc.gpsimd.dma_start(out=bk_sb, in_=bk[:, :])
        if has_bv:
            bv_sb = pp.tile([128, DPC], fp32)
            bv_ap = bv[:, :]
            nc.gpsimd.dma_start(
                out=bv_sb,
                in_=bass.AP(tensor=bv_ap.tensor, offset=bv_ap.offset,
                            ap=[[0, 128], bv_ap.ap[1]]),
            )

        p2 = ctx.enter_context(tc.tile_pool(name="ph2", bufs=1))
        ps = ctx.enter_context(tc.tile_pool(name="ps", bufs=1, space="PSUM"))
        at_tiles = {}
        vjb_iter = iter(range(NJB))

        def emit_qk_batch(ns, tags):
            chains = []
            for n in ns:
                for nm in ("q", "k"):
                    chains.append((nm, n))
            pqs = {}
            for (nm, n), tg in zip(chains, tags):
                pqs[(nm, n)] = ps.tile([128, 512], fp32, tag=tg,
                                       name=f"ps_{nm}{n}")
            for kc in range(KC):
                for nm, n in chains:
                    nc.tensor.matmul(
                        pqs[(nm, n)],
                        lhsT=w_sbs[nm][:, kc, :],
                        rhs=xT_sb[kc][:, n * 512:(n + 1) * 512],
                        start=(kc == 0),
                        stop=(kc == KC - 1),
                    )
            for nm, n in chains:
                dsts = qU if nm == "q" else kR
                bias_sb = None
                if nm == "q" and has_bq:
                    bias_sb = bq_sb
                if nm == "k" and has_bk:
                    bias_sb = bk_sb
                for h in range(HPC):
                    dst = dsts[h][0:64, n * 512:(n + 1) * 512]
                    sr = pqs[(nm, n)][h * 64:(h + 1) * 64, :]
                    if bias_sb is not None:
                        nc.vector.tensor_scalar_add(
                            dst, sr, bias_sb[h * 64:(h + 1) * 64, 0:1]
                        )
                    else:
                        nc.vector.tensor_copy(dst, sr)

        def emit_v(count):
            for vjb in [v for _, v in zip(range(count), vjb_iter)]:
                psv = ps.tile([128, DPC], fp32, tag=f"abcd"[vjb % 4],
                              name=f"psv{vjb}")
                for kc in range(KC):
                    nc.tensor.matmul(
                        psv,
                        lhsT=xT_sb[kc][:, vjb * 128:(vjb + 1) * 128],
                        rhs=w_sbs["v"][:, kc, :],
                        start=(kc == 0),
                        stop=(kc == KC - 1),
                    )
                dst = vp[vjb][:, :, 0:64]
                sr = psv[:, :].rearrange("p (h m) -> p h m", h=HPC)
                if has_bv:
                    bvr = bv_sb[:, :].rearrange("p (h m) -> p h m", h=HPC)
                    nc.vector.tensor_add(dst, sr, bvr)
                else:
                    nc.vector.tensor_copy(dst, sr)

        def emit_chunk(jb, ic):
            key = (jb, ic)
            at_tiles[key] = p2.tile(
                [128, HPC, 512], bf16, tag=f"at{jb}", bufs=2,
                name=f"at{jb}_{ic}")
            at = at_tiles[key]
            sc = ps.tile([128, HPC * 512], fp32, tag="sc", bufs=2,
                         name=f"sc{jb}_{ic}")
            for h in range(HPC):
                nc.tensor.matmul(
                    sc[:, h * 512:(h + 1) * 512],
                    lhsT=kR[h][:, jb * 128:(jb + 1) * 128],
                    rhs=qU[h][:, ic * 512:(ic + 1) * 512],
                    start=True,
                    stop=True,
                )
            scr = sc[:, :].rearrange("p (h n) -> p h n", h=HPC)
            nc.scalar.activation(at, scr, Exp)
            mt = p2.tile([128, 512], bf16, tag="mt", bufs=2,
                         name=f"mt{jb}_{ic}")
            nc.sync.dma_start(
                out=mt,
                in_=maskT[jb * 128:(jb + 1) * 128,
                          ic * 512:(ic + 1) * 512])
            mt_b2 = bass.AP(
                tensor=mt.tensor, offset=mt.offset,
                ap=[mt.ap[0], [0, HPC], mt.ap[1]],
            )
            nc.vector.tensor_mul(at, at, mt_b2)

        def emit_ctx(h, ic):
            jmax = NJB
            cps = ps.tile([65, 512], fp32, tag="ab"[h], name=f"cps{h}_{ic}")
            for jb in range(jmax):
                rhs = at_tiles[(jb, ic)][:, h, 0:512]
                nc.tensor.matmul(
                    cps,
                    lhsT=vp[jb][:, h, :],
                    rhs=rhs,
                    start=(jb == 0),
                    stop=(jb == jmax - 1),
                )
            rr = p2.tile([1, 512], fp32, tag="rr", bufs=2, name=f"rr{h}_{ic}")
            nc.vector.tensor_copy(rr, cps[64:65, :])
            rb = p2.tile([64, 512], fp32, tag="rb", bufs=2, name=f"rb{h}_{ic}")
            nc.gpsimd.partition_broadcast(rb, rr)
            nc.vector.reciprocal_approx_fast(rb, rb)
            nc.vector.tensor_mul(
                ctxT[ic][h * 64:(h + 1) * 64, :], cps[0:64, :], rb,
            )

        def emit_outproj(ib):
            ob = p2.tile([128, HID], bf16, tag="ob", bufs=3, name=f"ob{ib}")
            for oc in range(2):
                ops = ps.tile([128, 512], fp32, tag="cd"[oc],
                              name=f"ops{ib}_{oc}")
                nc.tensor.matmul(
                    ops,
                    lhsT=ctxT[ib // 4][:, (ib % 4) * 128:(ib % 4 + 1) * 128],
                    rhs=wo_sb[:, oc * 512:(oc + 1) * 512],
                    start=True,
                    stop=True,
                )
                nc.vector.tensor_copy(ob[:, oc * 512:(oc + 1) * 512], ops)
            nc.sync.dma_start(out=out[ib * 128:(ib + 1) * 128, :], in_=ob)

        emit_qk_batch([0, 1], ["a", "b", "c", "d"])
        emit_qk_batch([2, 3], ["a", "b", "c", "d"])
        emit_v(NJB)
        for ic in range(NIC):
            for jb in range(NJB):
                emit_chunk(jb, ic)
            for h in range(HPC):
                emit_ctx(h, ic)
            for ib in range(4 * ic, 4 * (ic + 1)):
                emit_outproj(ib)

    nc.compile()
    return nc



def kernel(x, Wq, bq, Wk, bk, Wv, bv, Wo, bo, cmw, mask, modality_info,
           _perf=None):
    from concourse.bass_utils import run_bass_kernel_spmd

    x = np.asarray(x, np.float32)
    Wq = np.asarray(Wq, np.float32)
    Wk = np.asarray(Wk, np.float32)
    Wv = np.asarray(Wv, np.float32)
    Wo = np.asarray(Wo, np.float32)
    bq_ = np.asarray(bq, np.float32)
    bk_ = np.asarray(bk, np.float32)
    bv_ = np.asarray(bv, np.float32)
    bo_ = np.asarray(bo, np.float32)
    cmw = np.asarray(cmw, np.float32)
    mask2 = np.asarray(mask)[0]
    mi = np.asarray(modality_info).astype(np.int64)[0]

    causal = bool(
        np.array_equal(mask2 != 0, np.tril(np.ones((S, S), bool)))
    )
    has_bq = bool(np.any(bq_))
    has_bk = bool(np.any(bk_))
    has_bv = bool(np.any(bv_))

    key = (causal, has_bq, has_bk, has_bv)
    if key not in _CACHE:
        if causal:
            _CACHE[key] = _build_causal(has_bq, has_bk, has_bv)
        else:
            _CACHE[key] = _build_noncausal(has_bq, has_bk, has_bv)
    nc = _CACHE[key]

    scale = 1.0 / math.sqrt(D)
    # rank-3 factorization of the gathered cross-modal bias
    R = np.zeros((S, 3), np.float32)
    R[np.arange(S), mi] = 1.0
    U = R @ cmw
    uT4 = np.zeros((4, S), BF16)
    rT4 = np.zeros((4, S), BF16)
    uT4[0:3, :] = U.T.astype(BF16)
    rT4[0:3, :] = R.T.astype(BF16)
    xTb = np.ascontiguousarray(x[0].T).astype(BF16)
    if causal:
        # n-major chunks: [NIC, 128, KC*512]
        xTn = np.ascontiguousarray(
            xTb.reshape(KC, 128, NIC, 512).transpose(2, 1, 0, 3)
            .reshape(NIC, 128, KC * 512))

    in_maps = []
    for c in range(NCORES):
        sl = slice(c * DPC, (c + 1) * DPC)
        m = {
            # scores scale folded into the q-side weights (and bias)
            "wqT": np.ascontiguousarray(Wq[sl, :].T * scale).astype(BF16),
            "wkT": np.ascontiguousarray(Wk[sl, :].T).astype(BF16),
            "wvT": np.ascontiguousarray(Wv[sl, :].T).astype(BF16),
            "woT": np.ascontiguousarray(Wo[:, sl].T).astype(BF16),
            "uT": uT4,
            "rT": rT4,
        }
        if causal:
            m["xTn"] = xTn
        else:
            m["xT"] = xTb
        if has_bq:
            m["bq"] = np.ascontiguousarray(bq_[sl, None] * scale)
        if has_bk:
            m["bk"] = np.ascontiguousarray(bk_[sl, None])
        if has_bv:
            m["bv"] = np.ascontiguousarray(bv_[None, sl])
        if not causal:
            m["maskT"] = np.ascontiguousarray(mask2.T != 0).astype(BF16)
        in_maps.append(m)

    res = run_bass_kernel_spmd(
        nc, in_maps, core_ids=list(range(NCORES)),
        trace=bool(_perf is not None),
    )
    outp = np.zeros((S, HID), np.float32)
    for r in res.results:
        outp += np.asarray(r["out"], dtype=np.float32)
    outp += bo_[None, :]
    if _perf is not None:
        _perf["exec_time_ns"] = res.exec_time_ns
        _perf["trace"] = res.instructions_and_trace
    return outp.reshape(B, S, HID)
